# revision 58
# baseline (speedup 1.0000x reference)
"""Dynamic-expert-conv kernel for Trainium2 (8 NeuronCores, SPMD data-parallel).

Problem: per-sample expert-mixed 3x3 conv:
    w[b] = sum_e attention[b,e] * weights[e]     # [O, C, 3, 3]
    out[b] = conv2d(input[b], w[b], pad=1) + bias_mix[b][:, None, None]

Strategy (per core, 4 samples), from HW microbenching (PE-only floor for
this matmul structure is ~197us; PSUM evacuation costs ~+20us on top
regardless of scheme; gpsimd tensor ops are ~10x slower than modeled and
must not be used for compute):
  - Expert weight bank resident in SBUF as bf16.
  - Per-sample combined weights built on DVE only, as a bf16 product/add
    tree: products via tensor_scalar_mul (4x DVE mode) and pairwise adds
    via tensor_tensor (2x mode) -- ~30us/sample, leaving DVE ~40% idle so
    it never gates the PE (the old fp32 stt chain was ~41us/sample).
  - Conv as accumulating bf16 PE matmuls, stationary reused across a
    group of y-tile PSUM banks: per (sample, o-chunk) two groups of 4+3
    banks, (c-chunk, offset) outer / y-tiles inner. Groups double-buffer
    PSUM: group A's banks evacuate while group B computes, so the PE
    never waits on a bank (single 7-bank sweeps measured slower on HW:
    the boundary evac drain stalls the PE and resets its p-state ramp).
  - PSUM evacuation (bias fused) on ScalarE activations (DVE helps only
    on the final group); output stores on the sync ring (the final
    groups spread stores over sync/scalar/gpsimd to cut the drain tail).
  - DMA rings: sync = att + j0-bank slices + output stores; scalar =
    j0-bank slices/bias/j1-bank behind the act-table load; gpsimd =
    input loads ONLY (so evacuations never queue behind a 5us input
    DMA). Input is bf16, host pre-padded to 58x58, streamed in two
    row-chunks split at the PSUM-group boundary.
  - Startup: the j=0 bank half is host-packed per combine-chunk slice
    ([p, k, e, w], one packed DMA per slice instead of 8 per-expert
    pieces that each pay ~500ns ring cost), and the combine runs
    per-slice (128/256/384/384 cols) just ahead of the PE, which issues
    its first matmul ~5us in instead of ~14us.
"""
import numpy as np

import concourse.bass as bass
import concourse.tile as tile
from concourse import bacc, mybir
from concourse.bass import ts
from concourse.bass_utils import run_bass_kernel_spmd
from contextlib import ExitStack

F32 = mybir.dt.float32
BF16 = mybir.dt.bfloat16
F8 = mybir.dt.float8e4
ADD = mybir.AluOpType.add
MULT = mybir.AluOpType.mult

import os
# tree: bf16 product/add tree on DVE; stt: fp32 scalar_tensor_tensor chain.
COMBINE_MODE = os.environ.get("COMBINE_MODE", "tree")

B, C, O, H, W, KK, E = 32, 256, 256, 56, 56, 3, 8
N_CORES = 8
B_LOC = B // N_CORES          # 4 samples per core
PW = H + 2                    # 58 padded
CCH = C // 128                # 2
OCH = O // 128                # 2
YT = 8                        # output rows per tile
NT = H // YT                  # 7 y-tiles
DYX = KK * KK                 # 9
QH = DYX * 128                # 1152 combined-weight cols per (c-chunk, o-chunk)

# y-tile groups sharing one stationary-load sweep (4+3 PSUM banks,
# double-buffered against each other)
GROUPS = [(0, 4), (4, 3)]


def _dedup_ldweights(nc):
    """Drop redundant PE weight reloads.

    Tile lowering splits every bf16 matmul into Ldweights+Matmult; the HW
    pays a serial ~54ns per LDWEIGHTS and does not skip reloads of the
    already-loaded stationary. conv_group orders matmuls so consecutive
    ones share the stationary, so any Ldweights whose access pattern
    matches the previous Ldweights in the same block's PE stream (with no
    waits/updates of its own) is a no-op and can be deleted: the weights
    are still in the array, and its (empty) sync carries nothing."""
    removed = 0
    for blk in nc.m.functions[0].blocks:
        last_key = None
        keep = []
        for inst in blk.instructions:
            if isinstance(inst, mybir.InstLdweights):
                si = inst.sync_info
                clean = si is None or (not si.on_wait and not si.on_update)
                key = (str(inst.ins[0]), str(inst.tile_position),
                       str(inst.perf_mode), str(inst.is_transpose))
                if clean and key == last_key:
                    removed += 1
                    continue
                last_key = key
            keep.append(inst)
        if removed:
            blk.instructions = keep
    return removed


def build(iters: int = 1):
    nc = bacc.Bacc("TRN2", target_bir_lowering=False, debug=False,
                   num_devices=N_CORES)
    x = nc.dram_tensor("x", [B_LOC, 128, CCH, PW, PW], BF16,
                       kind="ExternalInput").ap()
    # fp8 copy of the input for the tap-0 DoubleRow matmul: only the
    # top-left 56x56 of the padded image (all tap-0 reads), re-packed
    # contiguously so the matmul rhs flattens to [128, 2, 448].
    x8 = nc.dram_tensor("x8", [B_LOC, 128, CCH, H, W], F8,
                        kind="ExternalInput").ap()
    bank = nc.dram_tensor("bank", [E, 128, CCH, OCH, QH], BF16,
                          kind="ExternalInput").ap()
    # Host-packed copy of the j=0 half, split at the prologue-combine
    # chunk boundaries, each slice contiguous as [p, k, e, w]: every
    # startup-critical slice arrives as ONE packed DMA covering all 8
    # experts with >=512B descriptor runs (a small per-expert piece pays
    # a ~500ns ring cost regardless of size, so 8-way splits were
    # overhead-bound; an e-strided AP halves DMA bandwidth).
    # Ordered by PE consumption: bf16 taps d1.. first (d0's cols 0:128
    # feed only the fp8 DoubleRow tap, which runs LAST in each group).
    J0_SLICES = ((128, 256), (256, 640), (640, QH), (0, 128))
    j0_dram = [
        nc.dram_tensor(f"bank_j0s{i}", [128, CCH, E, hi - lo], BF16,
                       kind="ExternalInput").ap()
        for i, (lo, hi) in enumerate(J0_SLICES)]
    att = nc.dram_tensor("att", [128, B_LOC * E], F32,
                         kind="ExternalInput").ap()
    bias_t = nc.dram_tensor("bias_t", [128, OCH, E], F32,
                            kind="ExternalInput").ap()
    out = nc.dram_tensor("out", [B_LOC, 128, OCH, H, W], F32,
                         kind="ExternalOutput").ap()

    with ExitStack() as ctx:
        tc = ctx.enter_context(tile.TileContext(nc))
        const = ctx.enter_context(tc.tile_pool(name="const", bufs=1))
        bankp = ctx.enter_context(tc.tile_pool(name="bankp", bufs=1))
        combp = ctx.enter_context(tc.tile_pool(name="combp", bufs=2))
        comb8p = ctx.enter_context(tc.tile_pool(name="comb8p", bufs=2))
        tmpp = ctx.enter_context(tc.tile_pool(name="tmpp", bufs=16))
        sampp = ctx.enter_context(tc.tile_pool(name="sampp", bufs=2))
        samp8p = ctx.enter_context(tc.tile_pool(name="samp8p", bufs=2))
        stagep = ctx.enter_context(tc.tile_pool(name="stagep", bufs=8))
        psump = ctx.enter_context(tc.tile_pool(name="psump", bufs=8,
                                               space="PSUM"))

        # att first on the sync ring (combine needs it immediately); the
        # scalar ring opens with the act-table load for the evacuation
        # activations. bias_t rides later (not needed until ~14us).
        att_sb = const.tile([128, B_LOC * E], F32)
        nc.sync.dma_start(att_sb[:], att[:])
        bias_sb = const.tile([128, OCH, E], F32)

        # j=0 half lives in the packed per-slice layout; j=1 half keeps
        # the per-expert layout.
        j0_sb = [bankp.tile([128, CCH, E, hi - lo], BF16, name=f"j0s{i}")
                 for i, (lo, hi) in enumerate(J0_SLICES)]
        j1_sb = bankp.tile([128, E, CCH, QH], BF16)
        # The j0k0 quarter is the PE-start critical path: both rings
        # carry its slices first, in the order the chunked prologue
        # combine consumes them; j0k1 follows (needed ~12us in).
        j0_dmas = (
            (nc.sync, 0, 0), (nc.sync, 0, 1),
            (nc.scalar, 0, 2), (nc.sync, 0, 3),
            (nc.scalar, 1, 0), (nc.scalar, 1, 1),
            (nc.sync, 1, 2), (nc.scalar, 1, 3),
        )
        for ring, k, si in j0_dmas:
            ring.dma_start(j0_sb[si][:, k, :, :], j0_dram[si][:, k, :, :])
        nc.scalar.dma_start(bias_sb[:], bias_t[:])
        for k in range(CCH):
            for e in range(E):
                ring = nc.sync if e % 2 == 0 else nc.scalar
                ring.dma_start(j1_sb[:, e, k, :], bank[e][:, k, 1, :])

        bias_comb = const.tile([128, B_LOC, OCH], F32)
        bias_junk = const.tile([128, E], F32)

        def combine_bias(b):
            # bias_comb[p, b, j] = sum_e bias_t[p, j, e] * att[p, b*E+e]
            for j in range(OCH):
                nc.vector.scalar_tensor_tensor(
                    bias_junk[:], bias_sb[:, j, :], 1.0,
                    att_sb[:, b * E:(b + 1) * E],
                    op0=MULT, op1=MULT,
                    accum_out=bias_comb[:, b, j:j + 1])

        def combine_quarter(b, cb, j, k):
            """cb[:,k,j,:] = sum_e att[b,e]*bank[:,e,k,j,:], in bf16 on
            DVE: products via tensor_scalar_mul (4x mode), pairwise adds
            via tensor_tensor (2x). j=0 quarters run per packed slice
            (graduated sizes, so each is combined just before the PE
            consumes it -- the PE eats ~128 cols / 0.75us)."""
            a = [att_sb[:, b * E + e:b * E + e + 1] for e in range(E)]
            V = nc.vector
            # j=0 quarters are stored per slice. Sample 0's j=1 quarters
            # are also sliced: a full-quarter combine is ~7.5us of serial
            # DVE and s0/j1 would otherwise finish only just as the PE
            # reaches it. Later samples combine ~30us ahead, so they skip
            # the extra per-op overhead of slicing where possible.
            if j == 0:
                pieces = [(j0_sb[si], lo, hi)
                          for si, (lo, hi) in enumerate(J0_SLICES)]
            elif b == 0:
                pieces = [(None, lo, hi) for lo, hi in J0_SLICES]
            else:
                pieces = [(None, 0, QH)]
            if COMBINE_MODE == "stt":
                for src, lo_c, hi_c in pieces:
                    w = hi_c - lo_c
                    cf = tmpp.tile([128, QH], F32, name="tmpf")
                    dst = cf[:, 0:w]
                    bk = [(src[:, k, e, :] if src is not None
                           else j1_sb[:, e, k, lo_c:hi_c])
                          for e in range(E)]
                    V.tensor_scalar_mul(dst, bk[0], a[0])
                    for e in range(1, E):
                        V.scalar_tensor_tensor(
                            dst, bk[e], a[e], dst, op0=MULT, op1=ADD)
                    V.tensor_scalar_mul(cb[:, k, j, lo_c:hi_c], dst, 1.0)
                return
            t = [tmpp.tile([128, QH], BF16, name="tmp") for _ in range(8)]
            for src, lo_c, hi_c in pieces:
                w = hi_c - lo_c
                s = slice(0, w)
                bk = [(src[:, k, e, :] if src is not None
                       else j1_sb[:, e, k, lo_c:hi_c]) for e in range(E)]
                for e in range(E):
                    V.tensor_scalar_mul(t[e][:, s], bk[e], a[e])
                V.tensor_tensor(t[0][:, s], t[0][:, s], t[1][:, s], op=ADD)
                V.tensor_tensor(t[2][:, s], t[2][:, s], t[3][:, s], op=ADD)
                V.tensor_tensor(t[4][:, s], t[4][:, s], t[5][:, s], op=ADD)
                V.tensor_tensor(t[6][:, s], t[6][:, s], t[7][:, s], op=ADD)
                V.tensor_tensor(t[0][:, s], t[0][:, s], t[2][:, s], op=ADD)
                V.tensor_tensor(t[4][:, s], t[4][:, s], t[6][:, s], op=ADD)
                V.tensor_tensor(cb[:, k, j, lo_c:hi_c], t[0][:, s],
                                t[4][:, s], op=ADD)

        def combine_sample(b, cb, cb8, prologue=False):
            for j in range(OCH):
                for k in range(CCH):
                    combine_quarter(b, cb, j, k)
                    # fp8 copy of the tap-0 stationary for the DoubleRow
                    # matmul (reads only cols 0:128 = slice 0).
                    nc.vector.tensor_scalar_mul(
                        cb8[:, k, j, :], cb[:, k, j, 0:128], 1.0)
                if j == 0:
                    combine_bias(b)

        def load_samp(b):
            # Dedicated gpsimd ring: an input DMA must never delay the
            # evacuation activations (scalar ring) or output stores
            # (sync ring). Split at the PSUM-group row boundary so the
            # first group's matmuls (rows 0:34) start ~2us before the
            # whole sample has landed.
            samp = sampp.tile([128, CCH, PW, PW], BF16, name="samp")
            nc.gpsimd.dma_start(samp[:, :, 0:34, :], x[b][:, :, 0:34, :])
            nc.gpsimd.dma_start(samp[:, :, 34:PW, :], x[b][:, :, 34:PW, :])
            # fp8 copy for the tap-0 matmul, which runs LAST in each
            # group, so this is never startup-critical.
            samp8 = samp8p.tile([128, CCH, H, W], F8, name="samp8")
            nc.gpsimd.dma_start(samp8[:], x8[b][:])
            return samp, samp8

        def conv_group(j, t0, tn, comb, comb8, samp, samp8):
            """One stationary-load sweep over y-tiles [t0, t0+tn):
            (k,d) outer so consecutive matmuls share one combined-weight
            tile (_dedup_ldweights drops their redundant reloads) and
            stream tn x 448 moving columns per load; y-tiles accumulate
            in parallel PSUM banks. Taps d=1..8 run bf16 per c-chunk;
            tap d=0 runs LAST as a single fp8e4m3 DoubleRow matmul over
            both c-chunks at 2x rate (one tap in fp8 keeps the measured
            rel err at ~1.5e-2, inside the 2e-2 gate; every further tap
            would eat the remaining margin)."""
            psums = [psump.tile([128, YT, W], F32, name="psum")
                     for _ in range(tn)]
            for k in range(CCH):
                for d in range(1, DYX):
                    dy, dx = d // KK, d % KK
                    lhsT = comb[:, k, j, d * 128:(d + 1) * 128]
                    first = (k == 0 and d == 1)
                    for i in range(tn):
                        r0 = (t0 + i) * YT + dy
                        rhs = samp[:, k, r0:r0 + YT, dx:dx + W]
                        nc.tensor.matmul(psums[i][:], lhsT, rhs,
                                         start=first, stop=False)
            lhsT8 = comb8[:, :, j, :]
            for i in range(tn):
                r0 = (t0 + i) * YT
                rhs8 = samp8[:, :, r0:r0 + YT, :]
                nc.tensor.matmul(psums[i][:], lhsT8, rhs8,
                                 start=False, stop=True,
                                 perf_mode=mybir.MatmulPerfMode.DoubleRow)
            return psums

        def evac_group(b, j, t0, tn, psums, last=False):
            """PSUM -> SBUF fp32 with mixed bias fused, on ScalarE; the
            final groups spread stores over all 3 rings to cut the
            drain tail."""
            rings = ((nc.sync, nc.scalar, nc.gpsimd) if last
                     else (nc.sync,))
            for i in range(tn):
                stage = stagep.tile([128, YT, W], F32, name="stage")
                if last and i % 2 == 1:
                    # Final sweep only: DVE (idle once combining is done)
                    # halves the evac drain.
                    nc.vector.tensor_scalar_add(stage[:], psums[i][:],
                                                bias_comb[:, b, j:j + 1])
                else:
                    nc.scalar.activation(
                        stage[:], psums[i][:],
                        mybir.ActivationFunctionType.Identity,
                        bias=bias_comb[:, b, j:j + 1], scale=1.0)
                rings[i % len(rings)].dma_start(
                    out[b][:, j:j + 1, ts(t0 + i, YT), :], stage[:])

        def body():
            samps = {0: load_samp(0)}
            combs = {0: combp.tile([128, CCH, OCH, QH], BF16, name="comb")}
            comb8s = {0: comb8p.tile([128, CCH, OCH, 128], F8,
                                     name="comb8")}
            combine_sample(0, combs[0], comb8s[0], prologue=True)
            for b in range(B_LOC):
                for j in range(OCH):
                    last_j = (b == B_LOC - 1 and j == OCH - 1)
                    for t0, tn in GROUPS:
                        psums = conv_group(j, t0, tn, combs[b], comb8s[b],
                                           *samps[b])
                        evac_group(b, j, t0, tn, psums, last=last_j)
                    if j == 0 and b + 1 < B_LOC:
                        samps[b + 1] = load_samp(b + 1)
                        combs[b + 1] = combp.tile([128, CCH, OCH, QH],
                                                  BF16, name="comb")
                        comb8s[b + 1] = comb8p.tile([128, CCH, OCH, 128],
                                                    F8, name="comb8")
                        combine_sample(b + 1, combs[b + 1], comb8s[b + 1])

        if iters == 1:
            body()
        else:
            # On-device repeat loop — used only for slope-based HW timing.
            with tc.For_i(0, iters, 1, hint_engines=(mybir.EngineType.PE,)):
                body()

    _dedup_ldweights(nc)
    nc.compile()
    return nc


def prep_inputs(input, attention, weights, bias):
    """Host-side shard + layout prep. Returns per-core in_maps."""
    import ml_dtypes
    input = np.asarray(input, dtype=np.float32)
    attention = np.asarray(attention, dtype=np.float32)
    weights = np.asarray(weights, dtype=np.float32)
    bias = np.asarray(bias, dtype=np.float32)

    xp = np.zeros((B, CCH, 128, PW, PW), ml_dtypes.bfloat16)
    xp[:, :, :, 1:H + 1, 1:W + 1] = input.reshape(B, CCH, 128, H, W)
    xp = np.ascontiguousarray(xp.transpose(0, 2, 1, 3, 4))  # [B,128,CCH,PW,PW]
    # Tap-0 reads padded rows/cols 0:56 — repack that corner contiguously.
    xp8 = np.ascontiguousarray(xp[:, :, :, 0:H, 0:W]).astype(
        ml_dtypes.float8_e4m3)

    # weights [E, O, C, ky, kx] -> bank[e, p(c_lo), c_chunk, o_chunk, (d,o_lo)]
    wt = weights.transpose(0, 2, 3, 4, 1)                    # [E, C, ky, kx, O]
    wt = wt.reshape(E, CCH, 128, DYX, OCH, 128)              # [E,k,p,d,j,o]
    bank = np.ascontiguousarray(wt.transpose(0, 2, 1, 4, 3, 5)
                                ).reshape(E, 128, CCH, OCH, QH)
    bank = bank.astype(ml_dtypes.bfloat16)

    bias_tp = np.ascontiguousarray(
        bias.T.reshape(OCH, 128, E).transpose(1, 0, 2))      # [128, OCH, E]

    # Packed j=0 slices: bank [E, 128, CCH, OCH, QH] -> [128, CCH, E, w]
    # (must match build()'s J0_SLICES order exactly)
    j0_slices = ((128, 256), (256, 640), (640, QH), (0, 128))
    bank_j0s = [
        np.ascontiguousarray(bank[:, :, :, 0, lo:hi].transpose(1, 2, 0, 3))
        for lo, hi in j0_slices]

    in_maps = []
    for m in range(N_CORES):
        sl = slice(m * B_LOC, (m + 1) * B_LOC)
        att_m = np.ascontiguousarray(
            np.broadcast_to(attention[sl].reshape(1, B_LOC * E),
                            (128, B_LOC * E)))
        in_maps.append({
            "x": np.ascontiguousarray(xp[sl]),
            "x8": np.ascontiguousarray(xp8[sl]),
            "bank": bank,
            **{f"bank_j0s{i}": s for i, s in enumerate(bank_j0s)},
            "att": att_m,
            "bias_t": bias_tp,
        })
    return in_maps


def gather_output(results):
    """Per-core [B_LOC, 128, OCH, H, W] -> full [B, O, H, W]."""
    outs = []
    for m in range(N_CORES):
        o = results[m]["out"]  # [B_LOC, 128, OCH, H, W]
        outs.append(o.transpose(0, 2, 1, 3, 4).reshape(B_LOC, O, H, W))
    return np.concatenate(outs, axis=0)


_NC_CACHE = {}


def _get_nc():
    if "nc" not in _NC_CACHE:
        _NC_CACHE["nc"] = build()
    return _NC_CACHE["nc"]


def kernel(input, attention, weights, bias):
    nc = _get_nc()
    in_maps = prep_inputs(input, attention, weights, bias)
    res = run_bass_kernel_spmd(nc, in_maps, list(range(N_CORES)))
    return gather_output(res.results)


# revision 59
# speedup vs baseline: 1.1002x; 1.1002x over previous
"""Dynamic-expert-conv kernel for Trainium2 (8 NeuronCores, SPMD data-parallel).

Problem: per-sample expert-mixed 3x3 conv:
    w[b] = sum_e attention[b,e] * weights[e]     # [O, C, 3, 3]
    out[b] = conv2d(input[b], w[b], pad=1) + bias_mix[b][:, None, None]

Strategy (per core, 4 samples), from HW microbenching (PE-only floor for
this matmul structure is ~197us; PSUM evacuation costs ~+20us on top
regardless of scheme; gpsimd tensor ops are ~10x slower than modeled and
must not be used for compute):
  - Expert weight bank resident in SBUF as bf16.
  - Per-sample combined weights built on DVE only, as a bf16 product/add
    tree: products via tensor_scalar_mul (4x DVE mode) and pairwise adds
    via tensor_tensor (2x mode) -- ~30us/sample, leaving DVE ~40% idle so
    it never gates the PE (the old fp32 stt chain was ~41us/sample).
  - Conv as accumulating bf16 PE matmuls, stationary reused across a
    group of y-tile PSUM banks: per (sample, o-chunk) two groups of 4+3
    banks, (c-chunk, offset) outer / y-tiles inner. Groups double-buffer
    PSUM: group A's banks evacuate while group B computes, so the PE
    never waits on a bank (single 7-bank sweeps measured slower on HW:
    the boundary evac drain stalls the PE and resets its p-state ramp).
  - PSUM evacuation (bias fused) on ScalarE activations (DVE helps only
    on the final group); output stores on the sync ring (the final
    groups spread stores over sync/scalar/gpsimd to cut the drain tail).
  - DMA rings: sync = att + j0-bank slices + output stores; scalar =
    j0-bank slices/bias/j1-bank behind the act-table load; gpsimd =
    input loads ONLY (so evacuations never queue behind a 5us input
    DMA). Input is bf16, host pre-padded to 58x58, streamed in two
    row-chunks split at the PSUM-group boundary.
  - Startup: the j=0 bank half is host-packed per combine-chunk slice
    ([p, k, e, w], one packed DMA per slice instead of 8 per-expert
    pieces that each pay ~500ns ring cost), and the combine runs
    per-slice (128/256/384/384 cols) just ahead of the PE, which issues
    its first matmul ~5us in instead of ~14us.
"""
import numpy as np

import concourse.bass as bass
import concourse.tile as tile
from concourse import bacc, mybir
from concourse.bass import ts
from concourse.bass_utils import run_bass_kernel_spmd
from contextlib import ExitStack

F32 = mybir.dt.float32
BF16 = mybir.dt.bfloat16
F8 = mybir.dt.float8e4
ADD = mybir.AluOpType.add
MULT = mybir.AluOpType.mult

import os
# tree: bf16 product/add tree on DVE; stt: fp32 scalar_tensor_tensor chain.
COMBINE_MODE = os.environ.get("COMBINE_MODE", "tree")

B, C, O, H, W, KK, E = 32, 256, 256, 56, 56, 3, 8
N_CORES = 8
B_LOC = B // N_CORES          # 4 samples per core
PW = H + 2                    # 58 padded
CCH = C // 128                # 2
OCH = O // 128                # 2
YT = 8                        # output rows per tile
NT = H // YT                  # 7 y-tiles
DYX = KK * KK                 # 9
QH = DYX * 128                # 1152 combined-weight cols per (c-chunk, o-chunk)

# y-tile groups sharing one stationary-load sweep (4+3 PSUM banks,
# double-buffered against each other)
GROUPS = [(0, 4), (4, 3)]


def _dedup_ldweights(nc):
    """Drop redundant PE weight reloads.

    Tile lowering splits every bf16 matmul into Ldweights+Matmult; the HW
    pays a serial ~54ns per LDWEIGHTS and does not skip reloads of the
    already-loaded stationary. conv_group orders matmuls so consecutive
    ones share the stationary, so any Ldweights whose access pattern
    matches the previous Ldweights in the same block's PE stream (with no
    waits/updates of its own) is a no-op and can be deleted: the weights
    are still in the array, and its (empty) sync carries nothing."""
    removed = 0
    for blk in nc.m.functions[0].blocks:
        last_key = None
        keep = []
        for inst in blk.instructions:
            if isinstance(inst, mybir.InstLdweights):
                si = inst.sync_info
                clean = si is None or (not si.on_wait and not si.on_update)
                key = (str(inst.ins[0]), str(inst.tile_position),
                       str(inst.perf_mode), str(inst.is_transpose))
                if clean and key == last_key:
                    removed += 1
                    continue
                last_key = key
            keep.append(inst)
        if removed:
            blk.instructions = keep
    return removed


def build(iters: int = 1):
    nc = bacc.Bacc("TRN2", target_bir_lowering=False, debug=False,
                   num_devices=N_CORES)
    x = nc.dram_tensor("x", [B_LOC, 128, CCH, PW, PW], BF16,
                       kind="ExternalInput").ap()
    # fp8 copy of the input for the tap-0 DoubleRow matmul: only the
    # top-left 56x56 of the padded image (all tap-0 reads), re-packed
    # contiguously so the matmul rhs flattens to [128, 2, 448].
    x8 = nc.dram_tensor("x8", [B_LOC, 128, CCH, H, W], F8,
                        kind="ExternalInput").ap()
    bank = nc.dram_tensor("bank", [E, 128, CCH, OCH, QH], BF16,
                          kind="ExternalInput").ap()
    # Host-packed copy of the j=0 half, split at the prologue-combine
    # chunk boundaries, each slice contiguous as [p, k, e, w]: every
    # startup-critical slice arrives as ONE packed DMA covering all 8
    # experts with >=512B descriptor runs (a small per-expert piece pays
    # a ~500ns ring cost regardless of size, so 8-way splits were
    # overhead-bound; an e-strided AP halves DMA bandwidth).
    # Ordered by PE consumption: bf16 taps d1.. first (d0's cols 0:128
    # feed only the fp8 DoubleRow tap, which runs LAST in each group).
    J0_SLICES = ((128, 256), (256, 640), (640, QH), (0, 128))
    j0_dram = [
        nc.dram_tensor(f"bank_j0s{i}", [128, CCH, E, hi - lo], BF16,
                       kind="ExternalInput").ap()
        for i, (lo, hi) in enumerate(J0_SLICES)]
    att = nc.dram_tensor("att", [128, B_LOC * E], F32,
                         kind="ExternalInput").ap()
    bias_t = nc.dram_tensor("bias_t", [128, OCH, E], F32,
                            kind="ExternalInput").ap()
    out = nc.dram_tensor("out", [B_LOC, 128, OCH, H, W], F32,
                         kind="ExternalOutput").ap()

    with ExitStack() as ctx:
        tc = ctx.enter_context(tile.TileContext(nc))
        const = ctx.enter_context(tc.tile_pool(name="const", bufs=1))
        bankp = ctx.enter_context(tc.tile_pool(name="bankp", bufs=1))
        combp = ctx.enter_context(tc.tile_pool(name="combp", bufs=2))
        comb8p = ctx.enter_context(tc.tile_pool(name="comb8p", bufs=2))
        tmpp = ctx.enter_context(tc.tile_pool(name="tmpp", bufs=16))
        sampp = ctx.enter_context(tc.tile_pool(name="sampp", bufs=2))
        samp8p = ctx.enter_context(tc.tile_pool(name="samp8p", bufs=2))
        stagep = ctx.enter_context(tc.tile_pool(name="stagep", bufs=8))
        psump = ctx.enter_context(tc.tile_pool(name="psump", bufs=8,
                                               space="PSUM"))

        # att first on the sync ring (combine needs it immediately); the
        # scalar ring opens with the act-table load for the evacuation
        # activations. bias_t rides later (not needed until ~14us).
        att_sb = const.tile([128, B_LOC * E], F32)
        nc.sync.dma_start(att_sb[:], att[:])
        bias_sb = const.tile([128, OCH, E], F32)

        # j=0 half lives in the packed per-slice layout; j=1 half keeps
        # the per-expert layout.
        j0_sb = [bankp.tile([128, CCH, E, hi - lo], BF16, name=f"j0s{i}")
                 for i, (lo, hi) in enumerate(J0_SLICES)]
        j1_sb = bankp.tile([128, E, CCH, QH], BF16)
        # The j0k0 quarter is the PE-start critical path: both rings
        # carry its slices first, in the order the chunked prologue
        # combine consumes them; j0k1 follows (needed ~12us in).
        j0_dmas = (
            (nc.sync, 0, 0), (nc.sync, 0, 1),
            (nc.scalar, 0, 2), (nc.sync, 0, 3),
            (nc.scalar, 1, 0), (nc.scalar, 1, 1),
            (nc.sync, 1, 2), (nc.scalar, 1, 3),
        )
        for ring, k, si in j0_dmas:
            ring.dma_start(j0_sb[si][:, k, :, :], j0_dram[si][:, k, :, :])
        nc.scalar.dma_start(bias_sb[:], bias_t[:])
        for k in range(CCH):
            for e in range(E):
                ring = nc.sync if e % 2 == 0 else nc.scalar
                ring.dma_start(j1_sb[:, e, k, :], bank[e][:, k, 1, :])

        bias_comb = const.tile([128, B_LOC, OCH], F32)
        bias_junk = const.tile([128, E], F32)

        def combine_bias(b):
            # bias_comb[p, b, j] = sum_e bias_t[p, j, e] * att[p, b*E+e]
            for j in range(OCH):
                nc.vector.scalar_tensor_tensor(
                    bias_junk[:], bias_sb[:, j, :], 1.0,
                    att_sb[:, b * E:(b + 1) * E],
                    op0=MULT, op1=MULT,
                    accum_out=bias_comb[:, b, j:j + 1])

        def combine_quarter(b, cb, j, k):
            """cb[:,k,j,:] = sum_e att[b,e]*bank[:,e,k,j,:], in bf16 on
            DVE: products via tensor_scalar_mul (4x mode), pairwise adds
            via tensor_tensor (2x). j=0 quarters run per packed slice
            (graduated sizes, so each is combined just before the PE
            consumes it -- the PE eats ~128 cols / 0.75us)."""
            a = [att_sb[:, b * E + e:b * E + e + 1] for e in range(E)]
            V = nc.vector
            # j=0 quarters are stored per slice. Sample 0's j=1 quarters
            # are also sliced: a full-quarter combine is ~7.5us of serial
            # DVE and s0/j1 would otherwise finish only just as the PE
            # reaches it. Later samples combine ~30us ahead, so they skip
            # the extra per-op overhead of slicing where possible.
            if j == 0:
                pieces = [(j0_sb[si], lo, hi)
                          for si, (lo, hi) in enumerate(J0_SLICES)]
            else:
                pieces = [(None, lo, hi) for lo, hi in J0_SLICES]
            if COMBINE_MODE == "stt":
                for src, lo_c, hi_c in pieces:
                    w = hi_c - lo_c
                    cf = tmpp.tile([128, QH], F32, name="tmpf")
                    dst = cf[:, 0:w]
                    bk = [(src[:, k, e, :] if src is not None
                           else j1_sb[:, e, k, lo_c:hi_c])
                          for e in range(E)]
                    V.tensor_scalar_mul(dst, bk[0], a[0])
                    for e in range(1, E):
                        V.scalar_tensor_tensor(
                            dst, bk[e], a[e], dst, op0=MULT, op1=ADD)
                    V.tensor_scalar_mul(cb[:, k, j, lo_c:hi_c], dst, 1.0)
                return
            t = [tmpp.tile([128, QH], BF16, name="tmp") for _ in range(8)]
            for src, lo_c, hi_c in pieces:
                w = hi_c - lo_c
                s = slice(0, w)
                bk = [(src[:, k, e, :] if src is not None
                       else j1_sb[:, e, k, lo_c:hi_c]) for e in range(E)]
                for e in range(E):
                    V.tensor_scalar_mul(t[e][:, s], bk[e], a[e])
                V.tensor_tensor(t[0][:, s], t[0][:, s], t[1][:, s], op=ADD)
                V.tensor_tensor(t[2][:, s], t[2][:, s], t[3][:, s], op=ADD)
                V.tensor_tensor(t[4][:, s], t[4][:, s], t[5][:, s], op=ADD)
                V.tensor_tensor(t[6][:, s], t[6][:, s], t[7][:, s], op=ADD)
                V.tensor_tensor(t[0][:, s], t[0][:, s], t[2][:, s], op=ADD)
                V.tensor_tensor(t[4][:, s], t[4][:, s], t[6][:, s], op=ADD)
                V.tensor_tensor(cb[:, k, j, lo_c:hi_c], t[0][:, s],
                                t[4][:, s], op=ADD)

        def combine_sample(b, cb, cb8, prologue=False):
            for j in range(OCH):
                for k in range(CCH):
                    combine_quarter(b, cb, j, k)
                    # fp8 copy of the tap-0 stationary for the DoubleRow
                    # matmul (reads only cols 0:128 = slice 0).
                    nc.vector.tensor_scalar_mul(
                        cb8[:, k, j, :], cb[:, k, j, 0:128], 1.0)
                if j == 0:
                    combine_bias(b)

        def load_samp(b):
            # Dedicated gpsimd ring: an input DMA must never delay the
            # evacuation activations (scalar ring) or output stores
            # (sync ring). Split at the PSUM-group row boundary so the
            # first group's matmuls (rows 0:34) start ~2us before the
            # whole sample has landed.
            samp = sampp.tile([128, CCH, PW, PW], BF16, name="samp")
            nc.gpsimd.dma_start(samp[:, :, 0:34, :], x[b][:, :, 0:34, :])
            nc.gpsimd.dma_start(samp[:, :, 34:PW, :], x[b][:, :, 34:PW, :])
            # fp8 copy for the tap-0 matmul, which runs LAST in each
            # group, so this is never startup-critical.
            samp8 = samp8p.tile([128, CCH, H, W], F8, name="samp8")
            nc.gpsimd.dma_start(samp8[:], x8[b][:])
            return samp, samp8

        def conv_group(j, t0, tn, comb, comb8, samp, samp8):
            """One stationary-load sweep over y-tiles [t0, t0+tn):
            (k,d) outer so consecutive matmuls share one combined-weight
            tile (_dedup_ldweights drops their redundant reloads) and
            stream tn x 448 moving columns per load; y-tiles accumulate
            in parallel PSUM banks. Taps d=1..8 run bf16 per c-chunk;
            tap d=0 runs LAST as a single fp8e4m3 DoubleRow matmul over
            both c-chunks at 2x rate (one tap in fp8 keeps the measured
            rel err at ~1.5e-2, inside the 2e-2 gate; every further tap
            would eat the remaining margin)."""
            psums = [psump.tile([128, YT, W], F32, name="psum")
                     for _ in range(tn)]
            for k in range(CCH):
                for d in range(1, DYX):
                    dy, dx = d // KK, d % KK
                    lhsT = comb[:, k, j, d * 128:(d + 1) * 128]
                    first = (k == 0 and d == 1)
                    for i in range(tn):
                        r0 = (t0 + i) * YT + dy
                        rhs = samp[:, k, r0:r0 + YT, dx:dx + W]
                        nc.tensor.matmul(psums[i][:], lhsT, rhs,
                                         start=first, stop=False)
            lhsT8 = comb8[:, :, j, :]
            for i in range(tn):
                r0 = (t0 + i) * YT
                rhs8 = samp8[:, :, r0:r0 + YT, :]
                nc.tensor.matmul(psums[i][:], lhsT8, rhs8,
                                 start=False, stop=True,
                                 perf_mode=mybir.MatmulPerfMode.DoubleRow)
            return psums

        def evac_group(b, j, t0, tn, psums, last=False):
            """PSUM -> SBUF fp32 with mixed bias fused, on ScalarE; the
            final groups spread stores over all 3 rings to cut the
            drain tail."""
            rings = ((nc.sync, nc.scalar, nc.gpsimd) if last
                     else (nc.sync,))
            for i in range(tn):
                stage = stagep.tile([128, YT, W], F32, name="stage")
                if last and i % 2 == 1:
                    # Final sweep only: DVE (idle once combining is done)
                    # halves the evac drain.
                    nc.vector.tensor_scalar_add(stage[:], psums[i][:],
                                                bias_comb[:, b, j:j + 1])
                else:
                    nc.scalar.activation(
                        stage[:], psums[i][:],
                        mybir.ActivationFunctionType.Identity,
                        bias=bias_comb[:, b, j:j + 1], scale=1.0)
                rings[i % len(rings)].dma_start(
                    out[b][:, j:j + 1, ts(t0 + i, YT), :], stage[:])

        def body():
            samps = {0: load_samp(0)}
            combs = {0: combp.tile([128, CCH, OCH, QH], BF16, name="comb")}
            comb8s = {0: comb8p.tile([128, CCH, OCH, 128], F8,
                                     name="comb8")}
            combine_sample(0, combs[0], comb8s[0], prologue=True)
            for b in range(B_LOC):
                for j in range(OCH):
                    last_j = (b == B_LOC - 1 and j == OCH - 1)
                    for t0, tn in GROUPS:
                        psums = conv_group(j, t0, tn, combs[b], comb8s[b],
                                           *samps[b])
                        evac_group(b, j, t0, tn, psums, last=last_j)
                    if j == 0 and b + 1 < B_LOC:
                        samps[b + 1] = load_samp(b + 1)
                        combs[b + 1] = combp.tile([128, CCH, OCH, QH],
                                                  BF16, name="comb")
                        comb8s[b + 1] = comb8p.tile([128, CCH, OCH, 128],
                                                    F8, name="comb8")
                        combine_sample(b + 1, combs[b + 1], comb8s[b + 1])

        if iters == 1:
            body()
        else:
            # On-device repeat loop — used only for slope-based HW timing.
            with tc.For_i(0, iters, 1, hint_engines=(mybir.EngineType.PE,)):
                body()

    _dedup_ldweights(nc)
    nc.compile()
    return nc


def prep_inputs(input, attention, weights, bias):
    """Host-side shard + layout prep. Returns per-core in_maps."""
    import ml_dtypes
    input = np.asarray(input, dtype=np.float32)
    attention = np.asarray(attention, dtype=np.float32)
    weights = np.asarray(weights, dtype=np.float32)
    bias = np.asarray(bias, dtype=np.float32)

    xp = np.zeros((B, CCH, 128, PW, PW), ml_dtypes.bfloat16)
    xp[:, :, :, 1:H + 1, 1:W + 1] = input.reshape(B, CCH, 128, H, W)
    xp = np.ascontiguousarray(xp.transpose(0, 2, 1, 3, 4))  # [B,128,CCH,PW,PW]
    # Tap-0 reads padded rows/cols 0:56 — repack that corner contiguously.
    xp8 = np.ascontiguousarray(xp[:, :, :, 0:H, 0:W]).astype(
        ml_dtypes.float8_e4m3)

    # weights [E, O, C, ky, kx] -> bank[e, p(c_lo), c_chunk, o_chunk, (d,o_lo)]
    wt = weights.transpose(0, 2, 3, 4, 1)                    # [E, C, ky, kx, O]
    wt = wt.reshape(E, CCH, 128, DYX, OCH, 128)              # [E,k,p,d,j,o]
    bank = np.ascontiguousarray(wt.transpose(0, 2, 1, 4, 3, 5)
                                ).reshape(E, 128, CCH, OCH, QH)
    bank = bank.astype(ml_dtypes.bfloat16)

    bias_tp = np.ascontiguousarray(
        bias.T.reshape(OCH, 128, E).transpose(1, 0, 2))      # [128, OCH, E]

    # Packed j=0 slices: bank [E, 128, CCH, OCH, QH] -> [128, CCH, E, w]
    # (must match build()'s J0_SLICES order exactly)
    j0_slices = ((128, 256), (256, 640), (640, QH), (0, 128))
    bank_j0s = [
        np.ascontiguousarray(bank[:, :, :, 0, lo:hi].transpose(1, 2, 0, 3))
        for lo, hi in j0_slices]

    in_maps = []
    for m in range(N_CORES):
        sl = slice(m * B_LOC, (m + 1) * B_LOC)
        att_m = np.ascontiguousarray(
            np.broadcast_to(attention[sl].reshape(1, B_LOC * E),
                            (128, B_LOC * E)))
        in_maps.append({
            "x": np.ascontiguousarray(xp[sl]),
            "x8": np.ascontiguousarray(xp8[sl]),
            "bank": bank,
            **{f"bank_j0s{i}": s for i, s in enumerate(bank_j0s)},
            "att": att_m,
            "bias_t": bias_tp,
        })
    return in_maps


def gather_output(results):
    """Per-core [B_LOC, 128, OCH, H, W] -> full [B, O, H, W]."""
    outs = []
    for m in range(N_CORES):
        o = results[m]["out"]  # [B_LOC, 128, OCH, H, W]
        outs.append(o.transpose(0, 2, 1, 3, 4).reshape(B_LOC, O, H, W))
    return np.concatenate(outs, axis=0)


_NC_CACHE = {}


def _get_nc():
    if "nc" not in _NC_CACHE:
        _NC_CACHE["nc"] = build()
    return _NC_CACHE["nc"]


def kernel(input, attention, weights, bias):
    nc = _get_nc()
    in_maps = prep_inputs(input, attention, weights, bias)
    res = run_bass_kernel_spmd(nc, in_maps, list(range(N_CORES)))
    return gather_output(res.results)
